# revision 2
# baseline (speedup 1.0000x reference)
"""Trainium2 distributed kernel for ALRDLinear + 3-bit per-tensor fake-quant.

Reference computation (tokens=8192, in=4096, rank=1024, out=4096, f32):
    y   = input @ B_w.T                       # [tokens, rank]
    y_q = fake_quant(y)                       # per-tensor symmetric 3-bit
    out = y_q @ A_w.T + A_b                   # [tokens, out]

Distribution: data-parallel over tokens across 8 NeuronCores (1024 tok/core).
Weights replicated. The only cross-core dependency is the per-tensor amax,
done as a tiny [128] AllReduce(max).

Numerics: matmul1 runs as 3 accumulating fp16 matmul passes on hi/lo splits
(x = xh + xl, B = Bh + Bl; the lo*lo term is dropped) giving ~1e-6 abs error
in y. That precision is required: y feeds round(y/scale), and rounding-boundary
flips are amplified by the 3-bit step size. Matmul2 uses exact small-int q in
bf16 against bf16 A-weights pre-multiplied by scale.
"""

import numpy as np
import ml_dtypes

P = 128
TOK, IN_F, OUT_F, RANK = 8192, 4096, 4096, 1024
NCORES = 8
TPC = TOK // NCORES            # tokens per core
KT1 = IN_F // P                # 32 contraction tiles for matmul1
MR = RANK // P                 # 8 rank tiles
NT1 = TPC // 512               # 2 token column-tiles in matmul1
MT2 = TPC // P                 # 8 token row-tiles in matmul2
NT2 = OUT_F // 512             # 8 out-feature tiles

QMAX = 3.0
QMIN = -4.0
MAGIC = 1.5 * 2.0**23          # round-to-nearest-even integer trick

_CACHE = {}


def _build():
    import concourse.mybir as mybir
    import concourse.tile as tile
    from concourse import bacc
    from concourse import bass_isa

    nc = bacc.Bacc(None, target_bir_lowering=False, debug=False, num_devices=NCORES)
    f32, f16, bf16 = mybir.dt.float32, mybir.dt.float16, mybir.dt.bfloat16

    xh_d = nc.dram_tensor("xh", [IN_F, TPC], f16, kind="ExternalInput")
    xl_d = nc.dram_tensor("xl", [IN_F, TPC], f16, kind="ExternalInput")
    bh_d = nc.dram_tensor("bh", [P, MR, KT1, P], f16, kind="ExternalInput")
    bl_d = nc.dram_tensor("bl", [P, MR, KT1, P], f16, kind="ExternalInput")
    aw_d = nc.dram_tensor("aw", [P, NT2, MR, 512], bf16, kind="ExternalInput")
    bias_d = nc.dram_tensor("bias", [P, OUT_F], f32, kind="ExternalInput")
    out_d = nc.dram_tensor("out", [TPC, OUT_F], f32, kind="ExternalOutput")

    cc_in = nc.dram_tensor("cc_in", [P, 1], f32)
    cc_out = nc.dram_tensor("cc_out", [P, 1], f32, addr_space="Shared")

    ts = lambda i, s: slice(i * s, (i + 1) * s)

    with tile.TileContext(nc) as tc:
        with (
            tc.tile_pool(name="stats", bufs=1) as stats,
            tc.tile_pool(name="ypool", bufs=1) as ypool,
            tc.tile_pool(name="psum", bufs=3, space="PSUM") as psum,
        ):
            y_t = ypool.tile([P, MR, TPC], f32, tag="y")
            am_part = stats.tile([P, MR * NT1], f32, tag="am_part")
            am1 = stats.tile([P, 1], f32, tag="am1")
            am_b = stats.tile([P, 1], f32, tag="am_b")
            amg = stats.tile([P, 1], f32, tag="amg")
            scale_t = stats.tile([P, 1], f32, tag="scale")
            inv_t = stats.tile([P, 1], f32, tag="inv")

            # ---------------- phase 1: y.T = B @ x.T (fp16 3-pass) -------
            with (
                tc.tile_pool(name="xpool", bufs=1) as xpool,
                tc.tile_pool(name="bpool", bufs=2) as bpool,
            ):
                xh_t = xpool.tile([P, KT1, TPC], f16, tag="xh")
                xl_t = xpool.tile([P, KT1, TPC], f16, tag="xl")
                for k in range(KT1):
                    nc.sync.dma_start(xh_t[:, k], xh_d[ts(k, P), :])
                    nc.sync.dma_start(xl_t[:, k], xl_d[ts(k, P), :])

                for mr in range(MR):
                    bh_t = bpool.tile([P, KT1, P], f16, tag="bh")
                    bl_t = bpool.tile([P, KT1, P], f16, tag="bl")
                    nc.sync.dma_start(bh_t[:], bh_d[:, mr])
                    nc.sync.dma_start(bl_t[:], bl_d[:, mr])
                    for nt in range(NT1):
                        ps = psum.tile([P, 512], f32, tag="ps")
                        for k in range(KT1):
                            nc.tensor.matmul(
                                ps[:], bh_t[:, k], xh_t[:, k, ts(nt, 512)],
                                start=(k == 0), stop=False)
                            nc.tensor.matmul(
                                ps[:], bh_t[:, k], xl_t[:, k, ts(nt, 512)],
                                start=False, stop=False)
                            nc.tensor.matmul(
                                ps[:], bl_t[:, k], xh_t[:, k, ts(nt, 512)],
                                start=False, stop=(k == KT1 - 1))
                        nc.scalar.copy(y_t[:, mr, ts(nt, 512)], ps[:])
                        idx = mr * NT1 + nt
                        nc.vector.tensor_reduce(
                            am_part[:, idx : idx + 1], ps[:],
                            axis=mybir.AxisListType.X, op=mybir.AluOpType.max,
                            apply_absolute_value=True)

            # ---------------- amax all-reduce + scale ---------------------
            nc.vector.tensor_reduce(
                am1[:], am_part[:], axis=mybir.AxisListType.X,
                op=mybir.AluOpType.max)
            nc.gpsimd.partition_all_reduce(
                am_b[:], am1[:], channels=P, reduce_op=bass_isa.ReduceOp.max)
            nc.gpsimd.dma_start(cc_in[:, :], am_b[:])
            nc.gpsimd.collective_compute(
                "AllReduce", mybir.AluOpType.max,
                replica_groups=[list(range(NCORES))],
                ins=[cc_in.ap().opt()], outs=[cc_out.ap().opt()])
            nc.gpsimd.dma_start(amg[:], cc_out[:, :])
            # scale = max(amax, 1e-8) / QMAX ; inv = 1/scale
            nc.vector.tensor_scalar(
                scale_t[:], amg[:], 1e-8, float(np.float32(1.0 / QMAX)),
                mybir.AluOpType.max, mybir.AluOpType.mult)
            nc.vector.reciprocal(inv_t[:], scale_t[:])

            # ---------------- phase 2: quant + out = q @ (Aw*scale) + b --
            with (
                tc.tile_pool(name="qpool", bufs=1) as qpool,
                tc.tile_pool(name="tpool", bufs=2) as tpool,
                tc.tile_pool(name="apool", bufs=2) as apool,
                tc.tile_pool(name="opool", bufs=4) as opool,
                tc.tile_pool(name="biasp", bufs=1) as biasp,
            ):
                bias_t = biasp.tile([P, OUT_F], f32, tag="bias")
                nc.sync.dma_start(bias_t[:], bias_d[:, :])

                q_t = qpool.tile([P, MR, TPC], bf16, tag="q")
                for mt in range(MT2):
                    sl = ts(mt, P)
                    t1 = tpool.tile([P, MR, P], f32, tag="t1")
                    t2 = tpool.tile([P, MR, P], f32, tag="t2")
                    # t1 = y*inv + MAGIC  (RNE to integer in the f32 lattice)
                    nc.vector.tensor_scalar(
                        t1[:], y_t[:, :, sl], inv_t[:], MAGIC,
                        mybir.AluOpType.mult, mybir.AluOpType.add)
                    # t2 = min(t1 - MAGIC, QMAX)
                    nc.vector.tensor_scalar(
                        t2[:], t1[:], -MAGIC, QMAX,
                        mybir.AluOpType.add, mybir.AluOpType.min)
                    # q = max(t2, QMIN)  -> bf16 (exact small ints)
                    nc.vector.tensor_scalar(
                        q_t[:, :, sl], t2[:], QMIN, None, mybir.AluOpType.max)

                for nt in range(NT2):
                    a_t = apool.tile([P, MR, 512], bf16, tag="aw")
                    nc.sync.dma_start(a_t[:], aw_d[:, nt])
                    a_s = apool.tile([P, MR, 512], bf16, tag="aws")
                    nc.vector.tensor_scalar(
                        a_s[:], a_t[:], scale_t[:], None, mybir.AluOpType.mult)
                    for mt in range(MT2):
                        ps2 = psum.tile([P, 512], f32, tag="ps")
                        for kr in range(MR):
                            nc.tensor.matmul(
                                ps2[:], q_t[:, kr, ts(mt, P)], a_s[:, kr],
                                start=(kr == 0), stop=(kr == MR - 1))
                        o_t = opool.tile([P, 512], f32, tag="o")
                        nc.vector.tensor_tensor(
                            o_t[:], ps2[:], bias_t[:, ts(nt, 512)],
                            mybir.AluOpType.add)
                        nc.sync.dma_start(out_d[ts(mt, P), ts(nt, 512)], o_t[:])

    nc.compile()
    return nc


def _get_nc():
    if "nc" not in _CACHE:
        _CACHE["nc"] = _build()
    return _CACHE["nc"]


def kernel(input, B_w, A_w, A_b):
    from concourse import bass_utils

    nc = _get_nc()

    f32 = np.float32
    bf16 = ml_dtypes.bfloat16

    # Weights (replicated, pre-laid-out for the PE's [K-on-partitions] form).
    BwT = np.ascontiguousarray(B_w.astype(f32, copy=False).T)     # [IN_F, RANK]
    Bh = BwT.astype(np.float16)
    Bl = (BwT - Bh.astype(f32)).astype(np.float16)
    Bh = np.ascontiguousarray(Bh.reshape(KT1, P, MR, P).transpose(1, 2, 0, 3))
    Bl = np.ascontiguousarray(Bl.reshape(KT1, P, MR, P).transpose(1, 2, 0, 3))

    AwT = np.ascontiguousarray(A_w.astype(f32, copy=False).T)     # [RANK, OUT_F]
    Aw = np.ascontiguousarray(
        AwT.astype(bf16).reshape(MR, P, NT2, 512).transpose(1, 2, 0, 3))

    bias_rep = np.ascontiguousarray(
        np.broadcast_to(A_b.astype(f32, copy=False), (P, OUT_F)))

    in_maps = []
    for c in range(NCORES):
        xT = np.ascontiguousarray(input[c * TPC : (c + 1) * TPC].astype(f32, copy=False).T)
        xh = xT.astype(np.float16)
        xl = (xT - xh.astype(f32)).astype(np.float16)
        in_maps.append(
            {"xh": xh, "xl": xl, "bh": Bh, "bl": Bl, "aw": Aw, "bias": bias_rep}
        )

    res = bass_utils.run_bass_kernel_spmd(nc, in_maps, core_ids=list(range(NCORES)))
    out = np.concatenate([res.results[c]["out"] for c in range(NCORES)], axis=0)
    return out.astype(np.float32, copy=False)
